# revision 14
# baseline (speedup 1.0000x reference)
"""Trainium2 Bass kernel for nn_ChannelProcessing (FAN channel attention block).

Math (per batch element, all in the reference's fp32 semantics):
  q = x @ q_w.T                          [N, C]
  attn[c] = sigmoid( sum_n softmax_N(q)[n,c] * kpool[n, head(c)] ) * temp
            with kpool = mean_ch softmax_N(x)
  h = fc2( gelu( dwconv3x3( fc1(x) ) + dw_b ) ) + fc2_b    [N, C]
  h = LayerNorm_C(h) * ln_w + ln_b
  out[n,c] = h[n,c] * attn[c];   av = out.T per (head, ch)

Sharding: data-parallel over batch B=8 -> one NeuronCore each. Each core
runs the full block for its batch element; no collectives.

Layout strategy on-core: channel-major ("x^T") activations so softmax /
conv / fc1 run along the free axis; fc2 flips to token-major by using the
activations as the stationary matmul operand, which is what LayerNorm
needs. Matmuls run in float32r (fp32 storage, ~1e-4 matmul precision,
4x the fp32 PE rate). The depthwise 3x3 conv runs as 9 shifted
multiply-accumulates over a zero-padded [58x58] image per 128-channel
chunk: a few taps on the PE (diagonal stationary matrices, PSUM
accumulation) and the rest on the Vector engine (scalar_tensor_tensor
read-modify-write into the same PSUM tile).
"""
import os
import numpy as np
from contextlib import ExitStack

import concourse.bass as bass
import concourse.mybir as mybir
import concourse.tile as tile
from concourse.bass_utils import run_bass_kernel_spmd
from concourse.masks import make_identity

F32 = mybir.dt.float32
F32R = mybir.dt.float32r
BF16 = mybir.dt.bfloat16
AL = mybir.AluOpType
AF = mybir.ActivationFunctionType

B, N, C = 8, 3136, 768
HID, NH, CH = 3072, 8, 96
HI = WI = 56
KC = C // 128            # 6 channel chunks
MC = HID // 128          # 24 hidden chunks
NSL, SLT = 7, 448        # fc1/conv slices: 7 x 448 tokens (8 rows of 56)
ROWS = 8                 # spatial rows per slice
PW = WI + 2              # padded row width 58
LN_EPS = 1e-5

N_PE_TAPS = 5            # conv taps on PE; rest on DVE
TAPS = [(0, 0), (-1, 0), (1, 0), (0, -1), (0, 1), (-1, -1), (-1, 1), (1, -1), (1, 1)]

# token tiles for transpose-in and fc2/out phases
TT = [(i * 128, min(128, N - i * 128)) for i in range((N + 127) // 128)]

_CAP_BY_TYPE = {"InstEventSemaphore": 2}


def _legalize_waits(nc):
    """This walrus build accepts at most 1 sync wait per instruction (2 on
    EventSemaphore). Hoist excess waits emitted by Tile onto NoOps inserted
    just before the offender on the same engine queue."""
    cnt = 0
    for fn in nc.m.functions:
        for bb in fn.blocks:
            insts = bb.instructions  # live list
            i = 0
            while i < len(insts):
                inst = insts[i]
                si = inst.sync_info
                if si is None:
                    i += 1
                    continue
                waits = list(si.on_wait)
                cap = _CAP_BY_TYPE.get(type(inst).__name__, 1)
                if len(waits) <= cap:
                    i += 1
                    continue
                keep, excess = waits[-cap:], waits[:-cap]
                for w in excess:
                    cnt += 1
                    nop = mybir.InstNoOp(name=f"waitfix-{cnt}", engine=inst.engine)
                    nop.sync_info = mybir.SyncInfo(on_wait=[w], on_update=[])
                    insts.insert(i, nop)
                    i += 1
                inst.sync_info = mybir.SyncInfo(on_wait=keep, on_update=list(si.on_update))
                i += 1
    return nc


def _r(ap):
    return ap.bitcast(F32R)


def build_program():
    nc = bass.Bass("TRN2", target_bir_lowering=False, debug=False, num_devices=B)

    xb = nc.declare_dram_parameter("xb", [N, C], F32, isOutput=False)
    qwT = nc.declare_dram_parameter("qwT", [C, C], F32, isOutput=False)       # q_w.T
    fc1wT = nc.declare_dram_parameter("fc1wT", [MC, KC * 128, 128], F32, isOutput=False)
    fc2wT = nc.declare_dram_parameter("fc2wT", [HID, C], F32, isOutput=False)  # fc2_w.T
    dgd = nc.declare_dram_parameter("dgd", [N_PE_TAPS, MC, 128, 128], F32, isOutput=False) \
        if N_PE_TAPS else None
    dwv = nc.declare_dram_parameter("dwv", [MC, 128, 9], F32, isOutput=False)
    fc1b = nc.declare_dram_parameter("fc1b", [MC, 128], F32, isOutput=False)
    dwb = nc.declare_dram_parameter("dwb", [MC, 128], F32, isOutput=False)
    seld = nc.declare_dram_parameter("seld", [KC, 128, NH], F32, isOutput=False)
    gmat = nc.declare_dram_parameter("gmat", [KC, NH, 128], F32, isOutput=False)
    tempd = nc.declare_dram_parameter("tempd", [NH, 2], F32, isOutput=False)
    lnwd = nc.declare_dram_parameter("lnwd", [C], F32, isOutput=False)
    lnbd = nc.declare_dram_parameter("lnbd", [C], F32, isOutput=False)
    fc2bd = nc.declare_dram_parameter("fc2bd", [C], F32, isOutput=False)

    out_d = nc.declare_dram_parameter("out", [N, C], F32, isOutput=True)
    av_d = nc.declare_dram_parameter("av", [C, N], F32, isOutput=True)

    gscr = nc.dram_tensor("gscr", [MC, 128, N], F32)
    attn_scr = nc.dram_tensor("attn_scr", [C], F32)

    with tile.TileContext(nc) as tc, ExitStack() as ctx:
        persist = ctx.enter_context(tc.tile_pool(name="persist", bufs=1))
        ps_small = ctx.enter_context(tc.tile_pool(name="ps_small", bufs=2, space="PSUM"))

        ident = persist.tile([128, 128], F32, name="ident")
        make_identity(nc, ident[:])
        fc1b_t = persist.tile([128, MC], F32, name="fc1b_t")
        nc.sync.dma_start(fc1b_t[:], fc1b.ap().rearrange("m p -> p m"))
        dwb_t = persist.tile([128, MC], F32, name="dwb_t")
        nc.sync.dma_start(dwb_t[:], dwb.ap().rearrange("m p -> p m"))
        dwv_t = persist.tile([128, MC, 9], F32, name="dwv_t")
        nc.sync.dma_start(dwv_t[:], dwv.ap().rearrange("m p t -> p m t"))
        lnw_b = persist.tile([128, C], F32, name="lnw_b")
        lnb_b = persist.tile([128, C], F32, name="lnb_b")
        fc2b_b = persist.tile([128, C], F32, name="fc2b_b")
        for t, d in ((lnw_b, lnwd), (lnb_b, lnbd), (fc2b_b, fc2bd)):
            a = d.ap()
            nc.sync.dma_start(t[:], bass.AP(tensor=a.tensor, offset=a.offset,
                                            ap=[[0, 128]] + list(a.ap)))
        alpha_b = persist.tile([128, C], F32, name="alpha_b")
        beta_b = persist.tile([128, C], F32, name="beta_b")
        eps_t = persist.tile([128, 1], F32, name="eps_t")
        nc.vector.memset(eps_t[:], LN_EPS)
        sel_t = persist.tile([128, KC, NH], F32R, name="sel_t")
        nc.sync.dma_start(sel_t[:], _r(seld.ap().rearrange("c p h -> p c h")))
        gmat_t = persist.tile([NH, KC, 128], F32R, name="gmat_t")
        nc.sync.dma_start(gmat_t[:], _r(gmat.ap().rearrange("c h p -> h c p")))
        temp_t = persist.tile([NH, 2], F32R, name="temp_t")
        nc.sync.dma_start(temp_t[:], _r(tempd.ap()))
        # attention scalars, [128, KC] each
        dk6 = persist.tile([128, KC], F32, name="dk6")
        idk6 = persist.tile([128, KC], F32, name="idk6")
        dq42 = persist.tile([128, KC, NSL], F32, name="dq42")
        nq42 = persist.tile([128, KC, NSL], F32, name="nq42")
        dq6 = persist.tile([128, KC], F32, name="dq6")
        idq6 = persist.tile([128, KC], F32, name="idq6")
        nq6 = persist.tile([128, KC], F32, name="nq6")
        l6 = persist.tile([128, KC], F32, name="l6")
        s6 = persist.tile([128, KC], F32, name="s6")
        tmp6 = persist.tile([128, KC], F32, name="tmp6")
        attn6 = persist.tile([128, KC], F32, name="attn6")
        a6 = persist.tile([KC, 128], F32, name="a6")
        kpool_sb = persist.tile([NH, N], F32R, name="kpool_sb")

        with (
            tc.tile_pool(name="p_xw", bufs=1) as p_xw,
            tc.tile_pool(name="ps_AB", bufs=4, space="PSUM") as ps_AB,
        ):
            xT = p_xw.tile([128, KC, N], F32R, name="xT")
            qwT_sb = p_xw.tile([128, KC, C], F32R, name="qwT_sb")
            nc.sync.dma_start(qwT_sb[:], _r(qwT.ap().rearrange("(c p) m -> p c m", p=128)))

            KA = int(os.environ.get("KA", "4"))
            # ---- Phase A0: transpose x into xT (channel-major, f32r)
            with tc.tile_pool(name="p_xin", bufs=3) as p_xin:
                for (t0, tp) in (TT if KA >= 1 else []):
                    x_t = p_xin.tile([128, C], F32, name="x_t")
                    nc.sync.dma_start(x_t[:tp, :], xb.ap()[t0:t0 + tp, :])
                    for c in range(KC):
                        tps = ps_small.tile([128, 128], F32, name="tps")
                        nc.tensor.transpose(tps[:, :tp], x_t[:tp, c * 128:(c + 1) * 128],
                                            ident[:tp, :tp])
                        nc.scalar.copy(_r(xT[:, c, t0:t0 + tp]), tps[:, :tp])

            # ---- Phase A + B under shared pools
            with tc.tile_pool(name="p_attn", bufs=1) as p_attn:
                exp_q = p_attn.tile([128, KC, N], BF16, name="exp_q")
                sk_c = p_attn.tile([128, N], F32R, name="sk_c")
                kb_c = p_attn.tile([128, SLT], BF16, name="kb_c")
                junk = p_attn.tile([128, SLT], BF16, name="junk")

                # A1: k-softmax pooled. exp over x^T, row-sum, scale, sel-matmul
                nc.vector.memset(kpool_sb[:].bitcast(F32), 0.0)
                for c in (range(KC) if KA >= 2 else []):
                    nc.scalar.activation(_r(sk_c[:]), xT[:, c, :].bitcast(F32), AF.Exp,
                                         accum_out=dk6[:, c:c + 1])
                    nc.vector.reciprocal(idk6[:, c:c + 1], dk6[:, c:c + 1])
                    nc.vector.tensor_scalar_mul(_r(sk_c[:]), sk_c[:].bitcast(F32),
                                                idk6[:, c:c + 1])
                    for j in range(NSL):
                        kp = ps_AB.tile([NH, SLT], F32, name="kp", tag="mm")
                        nc.tensor.matmul(kp[:], sel_t[:, c, :],
                                         sk_c[:, j * SLT:(j + 1) * SLT],
                                         start=True, stop=True)
                        nc.vector.tensor_add(kpool_sb[:, j * SLT:(j + 1) * SLT],
                                             kpool_sb[:, j * SLT:(j + 1) * SLT].bitcast(F32),
                                             kp[:])

                # A2: q = x @ q_w.T (channel-major out), exp into bf16 + denominators
                for c in (range(KC) if KA >= 3 else []):
                    for j in range(NSL):
                        qp = ps_AB.tile([128, SLT], F32, name="qp", tag="mm")
                        for k in range(KC):
                            nc.tensor.matmul(qp[:], qwT_sb[:, k, c * 128:(c + 1) * 128],
                                             xT[:, k, j * SLT:(j + 1) * SLT],
                                             start=(k == 0), stop=(k == KC - 1))
                        nc.scalar.activation(exp_q[:, c, j * SLT:(j + 1) * SLT], qp[:],
                                             AF.Exp, accum_out=dq42[:, c, j:j + 1])
                nc.vector.tensor_reduce(dq6[:], dq42[:], axis=mybir.AxisListType.X,
                                        op=AL.add)
                nc.vector.reciprocal(idq6[:], dq6[:])

                # A3: num_q = sum_n exp_q * kpool[head]; then attn vector
                kpool_r = kpool_sb[:]
                for c in (range(KC) if KA >= 4 else []):
                    for j in range(NSL):
                        kbp = ps_AB.tile([128, SLT], F32, name="kbp", tag="mm")
                        nc.tensor.matmul(kbp[:], gmat_t[:, c, :],
                                         kpool_r[:, j * SLT:(j + 1) * SLT],
                                         start=True, stop=True)
                        nc.scalar.copy(kb_c[:], kbp[:])
                        nc.vector.scalar_tensor_tensor(
                            out=junk[:], in0=exp_q[:, c, j * SLT:(j + 1) * SLT],
                            scalar=1.0, in1=kb_c[:], op0=AL.mult, op1=AL.mult,
                            accum_out=nq42[:, c, j:j + 1])
                nc.vector.tensor_reduce(nq6[:], nq42[:], axis=mybir.AxisListType.X,
                                        op=AL.add)
                nc.vector.tensor_mul(l6[:], nq6[:], idq6[:])
                nc.scalar.activation(s6[:], l6[:], AF.Sigmoid)
                for c in range(KC):
                    tq = ps_small.tile([128, 128], F32, name="tps")
                    nc.tensor.matmul(tq[:, 0:2], gmat_t[:, c, :], temp_t[:],
                                     start=True, stop=True)
                    nc.scalar.copy(tmp6[:, c:c + 1], tq[:, 0:1])
                nc.vector.tensor_mul(attn6[:], s6[:], tmp6[:])
                # attn6 [128, KC] -> row vector via PE transpose + DRAM bounce
                atp = ps_small.tile([128, 128], F32, name="tps")
                nc.tensor.transpose(atp[:KC, :], attn6[:], ident[:])
                nc.scalar.copy(a6[:], atp[:KC, :])
                nc.sync.dma_start(attn_scr[:].rearrange("(a b) -> a b", a=KC), a6[:])
                arow = attn_scr[:]
                nc.sync.dma_start(alpha_b[:], bass.AP(tensor=arow.tensor, offset=arow.offset,
                                                      ap=[[0, 128]] + list(arow.ap)))
                nc.vector.tensor_mul(beta_b[:], alpha_b[:], lnb_b[:])
                nc.vector.tensor_mul(alpha_b[:], alpha_b[:], lnw_b[:])

            # ---- Phase B: fc1 -> pad -> conv3x3 -> gelu -> g scratch, per chunk
            with (
                tc.tile_pool(name="p_mlp", bufs=2) as p_mlp,
                tc.tile_pool(name="p_wstream", bufs=3) as p_wstream,
            ):
                for m in range(int(os.environ.get("KMC", MC))):
                    w1 = p_wstream.tile([128, KC, 128], F32R, name="w1")
                    nc.sync.dma_start(w1[:], _r(fc1wT.ap()[m].rearrange(
                        "(c p) m2 -> p c m2", p=128)))
                    if N_PE_TAPS:
                        dg = p_wstream.tile([128, N_PE_TAPS, 128], F32R, name="dg")
                        nc.sync.dma_start(dg[:], _r(dgd.ap()[:, m].rearrange(
                            "t p m2 -> p t m2")))
                    pm = p_mlp.tile([128, PW * PW], F32R, name="pm")
                    pmv = pm[:].rearrange("p (h w) -> p h w", h=PW)
                    # zero the one-cell border
                    pmvf = pm[:].bitcast(F32).rearrange("p (h w) -> p h w", h=PW)
                    nc.vector.memset(pmvf[:, 0, :], 0.0)
                    nc.vector.memset(pmvf[:, PW - 1, :], 0.0)
                    nc.vector.memset(pmvf[:, :, 0], 0.0)
                    nc.vector.memset(pmvf[:, :, PW - 1], 0.0)
                    nc.vector.tensor_copy(pmv[:, 0, :], pmvf[:, 0, :])
                    nc.vector.tensor_copy(pmv[:, PW - 1, :], pmvf[:, PW - 1, :])
                    nc.vector.tensor_copy(pmv[:, :, 0], pmvf[:, :, 0])
                    nc.vector.tensor_copy(pmv[:, :, PW - 1], pmvf[:, :, PW - 1])
                    gm = p_mlp.tile([128, N], F32R, name="gm")
                    for j in range(NSL):
                        fp = ps_AB.tile([128, SLT], F32, name="fp", tag="mm")
                        for k in range(KC):
                            nc.tensor.matmul(fp[:], w1[:, k, :],
                                             xT[:, k, j * SLT:(j + 1) * SLT],
                                             start=(k == 0), stop=(k == KC - 1))
                        nc.scalar.activation(
                            pmv[:, 1 + j * ROWS:1 + (j + 1) * ROWS, 1:1 + WI],
                            fp[:].rearrange("p (r w) -> p r w", r=ROWS),
                            AF.Identity, bias=fc1b_t[:, m:m + 1])
                    for j in range(NSL):
                        cp = ps_AB.tile([128, SLT], F32, name="cp", tag="mm")
                        cpv = cp[:].rearrange("p (r w) -> p r w", r=ROWS)
                        for t, (dy, dx) in enumerate(TAPS):
                            src = pmv[:, 1 + j * ROWS + dy:1 + (j + 1) * ROWS + dy,
                                      1 + dx:1 + dx + WI]
                            if t < N_PE_TAPS:
                                nc.tensor.matmul(cp[:], dg[:, t, :], src,
                                                 start=(t == 0),
                                                 stop=(t == N_PE_TAPS - 1))
                            else:
                                nc.vector.scalar_tensor_tensor(
                                    out=cpv, in0=src.bitcast(F32),
                                    scalar=dwv_t[:, m, t:t + 1], in1=cpv,
                                    op0=AL.mult, op1=AL.add)
                        nc.scalar.activation(_r(gm[:, j * SLT:(j + 1) * SLT]), cp[:],
                                             AF.Gelu, bias=dwb_t[:, m:m + 1])
                    nc.sync.dma_start(gscr[m], gm[:].bitcast(F32))

        # ---- Phase C: fc2 + LayerNorm + attn scale -> out, av
        with (
            tc.tile_pool(name="p_fc2w", bufs=1) as p_fc2w,
            tc.tile_pool(name="p_tok", bufs=2) as p_tok,
            tc.tile_pool(name="ps_C", bufs=3, space="PSUM") as ps_C,
        ):
            w2 = p_fc2w.tile([128, MC, C], F32R, name="w2")
            nc.sync.dma_start(w2[:], _r(fc2wT.ap().rearrange("(m p) n -> p m n", p=128)))
            for (t0, tp) in TT[:int(os.environ.get("KTT", len(TT)))]:
                gt = p_tok.tile([128, MC, 128], F32R, name="gt")
                nc.sync.dma_start(gt[:, :, :tp], _r(gscr[:, :, t0:t0 + tp].rearrange(
                    "m p n -> p m n")))
                hp = ps_C.tile([128, C], F32, name="hp")
                for m in range(MC):
                    nc.tensor.matmul(hp[:tp, :512], gt[:, m, :tp], w2[:, m, :512],
                                     start=(m == 0), stop=(m == MC - 1))
                for m in range(MC):
                    nc.tensor.matmul(hp[:tp, 512:], gt[:, m, :tp], w2[:, m, 512:],
                                     start=(m == 0), stop=(m == MC - 1))
                ht = p_tok.tile([128, C], F32, name="ht")
                nc.vector.scalar_tensor_tensor(out=ht[:tp, :], in0=hp[:tp, :], scalar=1.0,
                                               in1=fc2b_b[:tp, :], op0=AL.mult, op1=AL.add)
                nsum = p_tok.tile([128, 1], F32, name="nsum")
                nc.vector.tensor_reduce(nsum[:tp, :], ht[:tp, :],
                                        axis=mybir.AxisListType.X, op=AL.add, negate=True)
                nmu = p_tok.tile([128, 1], F32, name="nmu")
                nc.scalar.mul(nmu[:tp, :], nsum[:tp, :], 1.0 / C)
                cent = p_tok.tile([128, C], F32, name="cent")
                nc.scalar.activation(cent[:tp, :], ht[:tp, :], AF.Identity,
                                     bias=nmu[:tp, 0:1])
                junk2 = p_tok.tile([128, C], F32, name="junk2")
                ssq = p_tok.tile([128, 1], F32, name="ssq")
                nc.scalar.activation(junk2[:tp, :], cent[:tp, :], AF.Square,
                                     accum_out=ssq[:tp, :])
                sd = p_tok.tile([128, 1], F32, name="sd")
                nc.scalar.activation(sd[:tp, :], ssq[:tp, :], AF.Sqrt,
                                     bias=eps_t[:tp, 0:1], scale=1.0 / C)
                rstd = p_tok.tile([128, 1], F32, name="rstd")
                nc.vector.reciprocal(rstd[:tp, :], sd[:tp, :])
                ot = p_tok.tile([128, C], F32, name="ot")
                nc.vector.scalar_tensor_tensor(out=ot[:tp, :], in0=cent[:tp, :],
                                               scalar=rstd[:tp, 0:1], in1=alpha_b[:tp, :],
                                               op0=AL.mult, op1=AL.mult)
                nc.vector.tensor_add(ot[:tp, :], ot[:tp, :], beta_b[:tp, :])
                nc.sync.dma_start(out_d.ap()[t0:t0 + tp, :], ot[:tp, :])
                avt = p_tok.tile([128, KC, 128], F32, name="avt")
                for c in range(KC):
                    vp = ps_small.tile([128, 128], F32, name="tps")
                    nc.tensor.transpose(vp[:, :tp], ot[:tp, c * 128:(c + 1) * 128],
                                        ident[:tp, :tp])
                    nc.scalar.copy(avt[:, c, :tp], vp[:, :tp])
                    nc.sync.dma_start(av_d.ap()[c * 128:(c + 1) * 128, t0:t0 + tp],
                                      avt[:, c, :tp])

    _legalize_waits(nc)
    return nc


_NC_CACHE = None


def _get_program():
    global _NC_CACHE
    if _NC_CACHE is None:
        _NC_CACHE = build_program()
    return _NC_CACHE


def kernel(x, H, W, q_w, temperature, fc1_w, fc1_b, dw_w, dw_b, fc2_w, fc2_b,
           ln_w, ln_b):
    x = np.asarray(x, np.float32)
    q_w = np.asarray(q_w, np.float32)
    temperature = np.asarray(temperature, np.float32)
    fc1_w = np.asarray(fc1_w, np.float32)
    fc1_b = np.asarray(fc1_b, np.float32)
    dw_w = np.asarray(dw_w, np.float32)
    dw_b = np.asarray(dw_b, np.float32)
    fc2_w = np.asarray(fc2_w, np.float32)
    fc2_b = np.asarray(fc2_b, np.float32)
    ln_w = np.asarray(ln_w, np.float32)
    ln_b = np.asarray(ln_b, np.float32)
    assert x.shape == (B, N, C) and int(H) == HI and int(W) == WI

    # host-side parameter layout prep (one-time, O(weights))
    qwT = np.ascontiguousarray(q_w.T)                                   # [C, C]
    f = fc1_w.reshape(MC, 128, KC, 128)
    fc1wT = np.ascontiguousarray(np.transpose(f, (0, 2, 3, 1))).reshape(MC, KC * 128, 128)
    fc2wT = np.ascontiguousarray(fc2_w.T)                               # [HID, C]
    dwf = dw_w.reshape(HID, 9)
    dwv = np.ascontiguousarray(dwf.reshape(MC, 128, 9))
    # order dwv taps to match TAPS ordering: column t of dwv must be TAPS[t]
    tap_idx = [(dy + 1) * 3 + (dx + 1) for (dy, dx) in TAPS]
    dwv = np.ascontiguousarray(dwv[:, :, tap_idx])
    if N_PE_TAPS:
        dgd = np.zeros((N_PE_TAPS, MC, 128, 128), np.float32)
        ii = np.arange(128)
        for t in range(N_PE_TAPS):
            dgd[t, :, ii, ii] = 0.0  # placeholder, filled below
        for t in range(N_PE_TAPS):
            for m in range(MC):
                dgd[t, m, ii, ii] = dwv[m, :, t]
    heads = np.arange(C) // CH
    seld = np.zeros((KC, 128, NH), np.float32)
    gmat = np.zeros((KC, NH, 128), np.float32)
    for c in range(KC):
        h = heads[c * 128:(c + 1) * 128]
        seld[c, np.arange(128), h] = 1.0 / CH
        gmat[c, h, np.arange(128)] = 1.0
    ins_shared = {
        "qwT": qwT, "fc1wT": fc1wT, "fc2wT": fc2wT, "dwv": dwv,
        "fc1b": np.ascontiguousarray(fc1_b.reshape(MC, 128)),
        "dwb": np.ascontiguousarray(dw_b.reshape(MC, 128)),
        "seld": seld, "gmat": gmat,
        "tempd": np.ascontiguousarray(np.repeat(temperature.reshape(NH, 1), 2, axis=1)),
        "lnwd": ln_w, "lnbd": ln_b, "fc2bd": fc2_b,
    }
    if N_PE_TAPS:
        ins_shared["dgd"] = dgd
    in_maps = [dict(ins_shared, xb=np.ascontiguousarray(x[b])) for b in range(B)]

    nc = _get_program()
    res = run_bass_kernel_spmd(nc, in_maps, list(range(B)))
    out = np.stack([res.results[b]["out"] for b in range(B)])            # [B, N, C]
    av = np.stack([res.results[b]["av"].reshape(NH, CH, N) for b in range(B)])
    return out, av


# revision 15
# speedup vs baseline: 1.1079x; 1.1079x over previous
"""Trainium2 Bass kernel for nn_ChannelProcessing (FAN channel attention block).

Math (per batch element, all in the reference's fp32 semantics):
  q = x @ q_w.T                          [N, C]
  attn[c] = sigmoid( sum_n softmax_N(q)[n,c] * kpool[n, head(c)] ) * temp
            with kpool = mean_ch softmax_N(x)
  h = fc2( gelu( dwconv3x3( fc1(x) ) + dw_b ) ) + fc2_b    [N, C]
  h = LayerNorm_C(h) * ln_w + ln_b
  out[n,c] = h[n,c] * attn[c];   av = out.T per (head, ch)

Sharding: data-parallel over batch B=8 -> one NeuronCore each. Each core
runs the full block for its batch element; no collectives.

Layout strategy on-core: channel-major ("x^T") activations so softmax /
conv / fc1 run along the free axis; fc2 flips to token-major by using the
activations as the stationary matmul operand, which is what LayerNorm
needs. Matmuls run in float32r (fp32 storage, ~1e-4 matmul precision,
4x the fp32 PE rate). The depthwise 3x3 conv runs as 9 shifted
multiply-accumulates over a zero-padded [58x58] image per 128-channel
chunk: a few taps on the PE (diagonal stationary matrices, PSUM
accumulation) and the rest on the Vector engine (scalar_tensor_tensor
read-modify-write into the same PSUM tile).
"""
import os
import numpy as np
from contextlib import ExitStack

import concourse.bass as bass
import concourse.mybir as mybir
import concourse.tile as tile
from concourse.bass_utils import run_bass_kernel_spmd
from concourse.masks import make_identity

F32 = mybir.dt.float32
F32R = mybir.dt.float32r
BF16 = mybir.dt.bfloat16
AL = mybir.AluOpType
AF = mybir.ActivationFunctionType

B, N, C = 8, 3136, 768
HID, NH, CH = 3072, 8, 96
HI = WI = 56
KC = C // 128            # 6 channel chunks
MC = HID // 128          # 24 hidden chunks
NSL, SLT = 7, 448        # fc1/conv slices: 7 x 448 tokens (8 rows of 56)
ROWS = 8                 # spatial rows per slice
PW = WI + 2              # padded row width 58
LN_EPS = 1e-5

N_PE_TAPS = 7            # conv taps on PE; rest on DVE
TAPS = [(0, 0), (-1, 0), (1, 0), (0, -1), (0, 1), (-1, -1), (-1, 1), (1, -1), (1, 1)]

# token tiles for transpose-in and fc2/out phases
TT = [(i * 128, min(128, N - i * 128)) for i in range((N + 127) // 128)]

_CAP_BY_TYPE = {"InstEventSemaphore": 2}


def _legalize_waits(nc):
    """This walrus build accepts at most 1 sync wait per instruction (2 on
    EventSemaphore). Hoist excess waits emitted by Tile onto NoOps inserted
    just before the offender on the same engine queue."""
    cnt = 0
    for fn in nc.m.functions:
        for bb in fn.blocks:
            insts = bb.instructions  # live list
            i = 0
            while i < len(insts):
                inst = insts[i]
                si = inst.sync_info
                if si is None:
                    i += 1
                    continue
                waits = list(si.on_wait)
                cap = _CAP_BY_TYPE.get(type(inst).__name__, 1)
                if len(waits) <= cap:
                    i += 1
                    continue
                keep, excess = waits[-cap:], waits[:-cap]
                for w in excess:
                    cnt += 1
                    nop = mybir.InstNoOp(name=f"waitfix-{cnt}", engine=inst.engine)
                    nop.sync_info = mybir.SyncInfo(on_wait=[w], on_update=[])
                    insts.insert(i, nop)
                    i += 1
                inst.sync_info = mybir.SyncInfo(on_wait=keep, on_update=list(si.on_update))
                i += 1
    return nc


def _r(ap):
    return ap.bitcast(F32R)


def build_program():
    nc = bass.Bass("TRN2", target_bir_lowering=False, debug=False, num_devices=B)

    xb = nc.declare_dram_parameter("xb", [N, C], F32, isOutput=False)
    qwT = nc.declare_dram_parameter("qwT", [C, C], F32, isOutput=False)       # q_w.T
    fc1wT = nc.declare_dram_parameter("fc1wT", [MC, KC * 128, 128], F32, isOutput=False)
    fc2wT = nc.declare_dram_parameter("fc2wT", [HID, C], F32, isOutput=False)  # fc2_w.T
    dgd = nc.declare_dram_parameter("dgd", [N_PE_TAPS, MC, 128, 128], F32, isOutput=False) \
        if N_PE_TAPS else None
    dwv = nc.declare_dram_parameter("dwv", [MC, 128, 9], F32, isOutput=False)
    fc1b = nc.declare_dram_parameter("fc1b", [MC, 128], F32, isOutput=False)
    dwb = nc.declare_dram_parameter("dwb", [MC, 128], F32, isOutput=False)
    seld = nc.declare_dram_parameter("seld", [KC, 128, NH], F32, isOutput=False)
    gmat = nc.declare_dram_parameter("gmat", [KC, NH, 128], F32, isOutput=False)
    tempd = nc.declare_dram_parameter("tempd", [NH, 2], F32, isOutput=False)
    lnwd = nc.declare_dram_parameter("lnwd", [C], F32, isOutput=False)
    lnbd = nc.declare_dram_parameter("lnbd", [C], F32, isOutput=False)
    fc2bd = nc.declare_dram_parameter("fc2bd", [C], F32, isOutput=False)

    out_d = nc.declare_dram_parameter("out", [N, C], F32, isOutput=True)
    av_d = nc.declare_dram_parameter("av", [C, N], F32, isOutput=True)

    gscr = nc.dram_tensor("gscr", [MC, 128, N], F32)
    attn_scr = nc.dram_tensor("attn_scr", [C], F32)

    with tile.TileContext(nc) as tc, ExitStack() as ctx:
        persist = ctx.enter_context(tc.tile_pool(name="persist", bufs=1))
        ps_small = ctx.enter_context(tc.tile_pool(name="ps_small", bufs=2, space="PSUM"))

        ident = persist.tile([128, 128], F32, name="ident")
        make_identity(nc, ident[:])
        fc1b_t = persist.tile([128, MC], F32, name="fc1b_t")
        nc.sync.dma_start(fc1b_t[:], fc1b.ap().rearrange("m p -> p m"))
        dwb_t = persist.tile([128, MC], F32, name="dwb_t")
        nc.sync.dma_start(dwb_t[:], dwb.ap().rearrange("m p -> p m"))
        dwv_t = persist.tile([128, MC, 9], F32, name="dwv_t")
        nc.sync.dma_start(dwv_t[:], dwv.ap().rearrange("m p t -> p m t"))
        lnw_b = persist.tile([128, C], F32, name="lnw_b")
        lnb_b = persist.tile([128, C], F32, name="lnb_b")
        fc2b_b = persist.tile([128, C], F32, name="fc2b_b")
        for t, d in ((lnw_b, lnwd), (lnb_b, lnbd), (fc2b_b, fc2bd)):
            a = d.ap()
            nc.sync.dma_start(t[:], bass.AP(tensor=a.tensor, offset=a.offset,
                                            ap=[[0, 128]] + list(a.ap)))
        alpha_b = persist.tile([128, C], F32, name="alpha_b")
        beta_b = persist.tile([128, C], F32, name="beta_b")
        eps_t = persist.tile([128, 1], F32, name="eps_t")
        nc.vector.memset(eps_t[:], LN_EPS)
        sel_t = persist.tile([128, KC, NH], F32R, name="sel_t")
        nc.sync.dma_start(sel_t[:], _r(seld.ap().rearrange("c p h -> p c h")))
        gmat_t = persist.tile([NH, KC, 128], F32R, name="gmat_t")
        nc.sync.dma_start(gmat_t[:], _r(gmat.ap().rearrange("c h p -> h c p")))
        temp_t = persist.tile([NH, 2], F32R, name="temp_t")
        nc.sync.dma_start(temp_t[:], _r(tempd.ap()))
        # attention scalars, [128, KC] each
        dk6 = persist.tile([128, KC], F32, name="dk6")
        idk6 = persist.tile([128, KC], F32, name="idk6")
        dq42 = persist.tile([128, KC, NSL], F32, name="dq42")
        nq42 = persist.tile([128, KC, NSL], F32, name="nq42")
        dq6 = persist.tile([128, KC], F32, name="dq6")
        idq6 = persist.tile([128, KC], F32, name="idq6")
        nq6 = persist.tile([128, KC], F32, name="nq6")
        l6 = persist.tile([128, KC], F32, name="l6")
        s6 = persist.tile([128, KC], F32, name="s6")
        tmp6 = persist.tile([128, KC], F32, name="tmp6")
        attn6 = persist.tile([128, KC], F32, name="attn6")
        a6 = persist.tile([KC, 128], F32, name="a6")
        kpool_sb = persist.tile([NH, N], F32R, name="kpool_sb")

        with (
            tc.tile_pool(name="p_xw", bufs=1) as p_xw,
            tc.tile_pool(name="ps_AB", bufs=3, space="PSUM") as ps_AB,
        ):
            xT = p_xw.tile([128, KC, N], F32R, name="xT")
            qwT_sb = p_xw.tile([128, KC, C], F32R, name="qwT_sb")
            nc.sync.dma_start(qwT_sb[:], _r(qwT.ap().rearrange("(c p) m -> p c m", p=128)))

            KA = int(os.environ.get("KA", "4"))
            # ---- Phase A0: transpose x into xT (channel-major, f32r)
            with tc.tile_pool(name="p_xin", bufs=3) as p_xin:
                for (t0, tp) in (TT if KA >= 1 else []):
                    x_t = p_xin.tile([128, C], F32, name="x_t")
                    nc.sync.dma_start(x_t[:tp, :], xb.ap()[t0:t0 + tp, :])
                    for c in range(KC):
                        tps = ps_small.tile([128, 128], F32, name="tps")
                        nc.tensor.transpose(tps[:, :tp], x_t[:tp, c * 128:(c + 1) * 128],
                                            ident[:tp, :tp])
                        nc.scalar.copy(_r(xT[:, c, t0:t0 + tp]), tps[:, :tp])

            # ---- Phase A + B under shared pools
            with tc.tile_pool(name="p_attn", bufs=1) as p_attn:
                exp_q = p_attn.tile([128, KC, N], BF16, name="exp_q")
                sk_c = p_attn.tile([128, N], F32R, name="sk_c")
                kb_c = p_attn.tile([128, SLT], BF16, name="kb_c")
                junk = p_attn.tile([128, SLT], BF16, name="junk")

                # A1: k-softmax pooled. exp over x^T, row-sum, scale, sel-matmul
                nc.vector.memset(kpool_sb[:].bitcast(F32), 0.0)
                for c in (range(KC) if KA >= 2 else []):
                    nc.scalar.activation(_r(sk_c[:]), xT[:, c, :].bitcast(F32), AF.Exp,
                                         accum_out=dk6[:, c:c + 1])
                    nc.vector.reciprocal(idk6[:, c:c + 1], dk6[:, c:c + 1])
                    nc.vector.tensor_scalar_mul(_r(sk_c[:]), sk_c[:].bitcast(F32),
                                                idk6[:, c:c + 1])
                    for j in range(NSL):
                        kp = ps_AB.tile([NH, SLT], F32, name="kp", tag="mm")
                        nc.tensor.matmul(kp[:], sel_t[:, c, :],
                                         sk_c[:, j * SLT:(j + 1) * SLT],
                                         start=True, stop=True)
                        nc.vector.tensor_add(kpool_sb[:, j * SLT:(j + 1) * SLT],
                                             kpool_sb[:, j * SLT:(j + 1) * SLT].bitcast(F32),
                                             kp[:])

                # A2: q = x @ q_w.T (channel-major out), exp into bf16 + denominators
                for c in (range(KC) if KA >= 3 else []):
                    for j in range(NSL):
                        qp = ps_AB.tile([128, SLT], F32, name="qp", tag="mm")
                        for k in range(KC):
                            nc.tensor.matmul(qp[:], qwT_sb[:, k, c * 128:(c + 1) * 128],
                                             xT[:, k, j * SLT:(j + 1) * SLT],
                                             start=(k == 0), stop=(k == KC - 1))
                        nc.scalar.activation(exp_q[:, c, j * SLT:(j + 1) * SLT], qp[:],
                                             AF.Exp, accum_out=dq42[:, c, j:j + 1])
                nc.vector.tensor_reduce(dq6[:], dq42[:], axis=mybir.AxisListType.X,
                                        op=AL.add)
                nc.vector.reciprocal(idq6[:], dq6[:])

                # A3: num_q = sum_n exp_q * kpool[head]; then attn vector
                kpool_r = kpool_sb[:]
                for c in (range(KC) if KA >= 4 else []):
                    for j in range(NSL):
                        kbp = ps_AB.tile([128, SLT], F32, name="kbp", tag="mm")
                        nc.tensor.matmul(kbp[:], gmat_t[:, c, :],
                                         kpool_r[:, j * SLT:(j + 1) * SLT],
                                         start=True, stop=True)
                        nc.scalar.copy(kb_c[:], kbp[:])
                        nc.vector.scalar_tensor_tensor(
                            out=junk[:], in0=exp_q[:, c, j * SLT:(j + 1) * SLT],
                            scalar=1.0, in1=kb_c[:], op0=AL.mult, op1=AL.mult,
                            accum_out=nq42[:, c, j:j + 1])
                nc.vector.tensor_reduce(nq6[:], nq42[:], axis=mybir.AxisListType.X,
                                        op=AL.add)
                nc.vector.tensor_mul(l6[:], nq6[:], idq6[:])
                nc.scalar.activation(s6[:], l6[:], AF.Sigmoid)
                for c in range(KC):
                    tq = ps_small.tile([128, 128], F32, name="tps")
                    nc.tensor.matmul(tq[:, 0:2], gmat_t[:, c, :], temp_t[:],
                                     start=True, stop=True)
                    nc.scalar.copy(tmp6[:, c:c + 1], tq[:, 0:1])
                nc.vector.tensor_mul(attn6[:], s6[:], tmp6[:])
                # attn6 [128, KC] -> row vector via PE transpose + DRAM bounce
                atp = ps_small.tile([128, 128], F32, name="tps")
                nc.tensor.transpose(atp[:KC, :], attn6[:], ident[:])
                nc.scalar.copy(a6[:], atp[:KC, :])
                nc.sync.dma_start(attn_scr[:].rearrange("(a b) -> a b", a=KC), a6[:])
                arow = attn_scr[:]
                nc.sync.dma_start(alpha_b[:], bass.AP(tensor=arow.tensor, offset=arow.offset,
                                                      ap=[[0, 128]] + list(arow.ap)))
                nc.vector.tensor_mul(beta_b[:], alpha_b[:], lnb_b[:])
                nc.vector.tensor_mul(alpha_b[:], alpha_b[:], lnw_b[:])

            # ---- Phase B: fc1 -> pad -> conv3x3 -> gelu -> g scratch, per chunk
            with (
                tc.tile_pool(name="p_mlp", bufs=2) as p_mlp,
                tc.tile_pool(name="p_wstream", bufs=3) as p_wstream,
            ):
                for m in range(int(os.environ.get("KMC", MC))):
                    w1 = p_wstream.tile([128, KC, 128], F32R, name="w1")
                    nc.sync.dma_start(w1[:], _r(fc1wT.ap()[m].rearrange(
                        "(c p) m2 -> p c m2", p=128)))
                    if N_PE_TAPS:
                        dg = p_wstream.tile([128, N_PE_TAPS, 128], F32R, name="dg")
                        nc.sync.dma_start(dg[:], _r(dgd.ap()[:, m].rearrange(
                            "t p m2 -> p t m2")))
                    pm = p_mlp.tile([128, PW * PW], F32R, name="pm")
                    pmv = pm[:].rearrange("p (h w) -> p h w", h=PW)
                    # zero the one-cell border
                    pmvf = pm[:].bitcast(F32).rearrange("p (h w) -> p h w", h=PW)
                    nc.vector.memset(pmvf[:, 0, :], 0.0)
                    nc.vector.memset(pmvf[:, PW - 1, :], 0.0)
                    nc.vector.memset(pmvf[:, :, 0], 0.0)
                    nc.vector.memset(pmvf[:, :, PW - 1], 0.0)
                    nc.vector.tensor_copy(pmv[:, 0, :], pmvf[:, 0, :])
                    nc.vector.tensor_copy(pmv[:, PW - 1, :], pmvf[:, PW - 1, :])
                    nc.vector.tensor_copy(pmv[:, :, 0], pmvf[:, :, 0])
                    nc.vector.tensor_copy(pmv[:, :, PW - 1], pmvf[:, :, PW - 1])
                    gm = p_mlp.tile([128, N], F32R, name="gm")
                    for j in range(NSL):
                        fp = ps_AB.tile([128, SLT], F32, name="fp", tag="mm")
                        for k in range(KC):
                            nc.tensor.matmul(fp[:], w1[:, k, :],
                                             xT[:, k, j * SLT:(j + 1) * SLT],
                                             start=(k == 0), stop=(k == KC - 1))
                        nc.scalar.activation(
                            pmv[:, 1 + j * ROWS:1 + (j + 1) * ROWS, 1:1 + WI],
                            fp[:].rearrange("p (r w) -> p r w", r=ROWS),
                            AF.Identity, bias=fc1b_t[:, m:m + 1])
                    for j in range(NSL):
                        cp = ps_AB.tile([128, SLT], F32, name="cp", tag="cv")
                        cpv = cp[:].rearrange("p (r w) -> p r w", r=ROWS)
                        for t, (dy, dx) in enumerate(TAPS):
                            src = pmv[:, 1 + j * ROWS + dy:1 + (j + 1) * ROWS + dy,
                                      1 + dx:1 + dx + WI]
                            if t < N_PE_TAPS:
                                nc.tensor.matmul(cp[:], dg[:, t, :], src,
                                                 start=(t == 0),
                                                 stop=(t == N_PE_TAPS - 1))
                            else:
                                nc.vector.scalar_tensor_tensor(
                                    out=cpv, in0=src.bitcast(F32),
                                    scalar=dwv_t[:, m, t:t + 1], in1=cpv,
                                    op0=AL.mult, op1=AL.add)
                        nc.scalar.activation(_r(gm[:, j * SLT:(j + 1) * SLT]), cp[:],
                                             AF.Gelu, bias=dwb_t[:, m:m + 1])
                    nc.sync.dma_start(gscr[m], gm[:].bitcast(F32))

        # ---- Phase C: fc2 + LayerNorm + attn scale -> out, av
        with (
            tc.tile_pool(name="p_fc2w", bufs=1) as p_fc2w,
            tc.tile_pool(name="p_tok", bufs=2) as p_tok,
            tc.tile_pool(name="ps_C", bufs=3, space="PSUM") as ps_C,
        ):
            w2 = p_fc2w.tile([128, MC, C], F32R, name="w2")
            nc.sync.dma_start(w2[:], _r(fc2wT.ap().rearrange("(m p) n -> p m n", p=128)))
            for (t0, tp) in TT[:int(os.environ.get("KTT", len(TT)))]:
                gt = p_tok.tile([128, MC, 128], F32R, name="gt")
                nc.sync.dma_start(gt[:, :, :tp], _r(gscr[:, :, t0:t0 + tp].rearrange(
                    "m p n -> p m n")))
                hp = ps_C.tile([128, C], F32, name="hp")
                for m in range(MC):
                    nc.tensor.matmul(hp[:tp, :512], gt[:, m, :tp], w2[:, m, :512],
                                     start=(m == 0), stop=(m == MC - 1))
                for m in range(MC):
                    nc.tensor.matmul(hp[:tp, 512:], gt[:, m, :tp], w2[:, m, 512:],
                                     start=(m == 0), stop=(m == MC - 1))
                ht = p_tok.tile([128, C], F32, name="ht")
                nc.vector.scalar_tensor_tensor(out=ht[:tp, :], in0=hp[:tp, :], scalar=1.0,
                                               in1=fc2b_b[:tp, :], op0=AL.mult, op1=AL.add)
                nsum = p_tok.tile([128, 1], F32, name="nsum")
                nc.vector.tensor_reduce(nsum[:tp, :], ht[:tp, :],
                                        axis=mybir.AxisListType.X, op=AL.add, negate=True)
                nmu = p_tok.tile([128, 1], F32, name="nmu")
                nc.scalar.mul(nmu[:tp, :], nsum[:tp, :], 1.0 / C)
                cent = p_tok.tile([128, C], F32, name="cent")
                nc.scalar.activation(cent[:tp, :], ht[:tp, :], AF.Identity,
                                     bias=nmu[:tp, 0:1])
                junk2 = p_tok.tile([128, C], F32, name="junk2")
                ssq = p_tok.tile([128, 1], F32, name="ssq")
                nc.scalar.activation(junk2[:tp, :], cent[:tp, :], AF.Square,
                                     accum_out=ssq[:tp, :])
                sd = p_tok.tile([128, 1], F32, name="sd")
                nc.scalar.activation(sd[:tp, :], ssq[:tp, :], AF.Sqrt,
                                     bias=eps_t[:tp, 0:1], scale=1.0 / C)
                rstd = p_tok.tile([128, 1], F32, name="rstd")
                nc.vector.reciprocal(rstd[:tp, :], sd[:tp, :])
                ot = p_tok.tile([128, C], F32, name="ot")
                nc.vector.scalar_tensor_tensor(out=ot[:tp, :], in0=cent[:tp, :],
                                               scalar=rstd[:tp, 0:1], in1=alpha_b[:tp, :],
                                               op0=AL.mult, op1=AL.mult)
                nc.vector.tensor_add(ot[:tp, :], ot[:tp, :], beta_b[:tp, :])
                nc.sync.dma_start(out_d.ap()[t0:t0 + tp, :], ot[:tp, :])
                avt = p_tok.tile([128, KC, 128], F32, name="avt")
                for c in range(KC):
                    vp = ps_small.tile([128, 128], F32, name="tps")
                    nc.tensor.transpose(vp[:, :tp], ot[:tp, c * 128:(c + 1) * 128],
                                        ident[:tp, :tp])
                    nc.scalar.copy(avt[:, c, :tp], vp[:, :tp])
                    nc.sync.dma_start(av_d.ap()[c * 128:(c + 1) * 128, t0:t0 + tp],
                                      avt[:, c, :tp])

    _legalize_waits(nc)
    return nc


_NC_CACHE = None


def _get_program():
    global _NC_CACHE
    if _NC_CACHE is None:
        _NC_CACHE = build_program()
    return _NC_CACHE


def kernel(x, H, W, q_w, temperature, fc1_w, fc1_b, dw_w, dw_b, fc2_w, fc2_b,
           ln_w, ln_b):
    x = np.asarray(x, np.float32)
    q_w = np.asarray(q_w, np.float32)
    temperature = np.asarray(temperature, np.float32)
    fc1_w = np.asarray(fc1_w, np.float32)
    fc1_b = np.asarray(fc1_b, np.float32)
    dw_w = np.asarray(dw_w, np.float32)
    dw_b = np.asarray(dw_b, np.float32)
    fc2_w = np.asarray(fc2_w, np.float32)
    fc2_b = np.asarray(fc2_b, np.float32)
    ln_w = np.asarray(ln_w, np.float32)
    ln_b = np.asarray(ln_b, np.float32)
    assert x.shape == (B, N, C) and int(H) == HI and int(W) == WI

    # host-side parameter layout prep (one-time, O(weights))
    qwT = np.ascontiguousarray(q_w.T)                                   # [C, C]
    f = fc1_w.reshape(MC, 128, KC, 128)
    fc1wT = np.ascontiguousarray(np.transpose(f, (0, 2, 3, 1))).reshape(MC, KC * 128, 128)
    fc2wT = np.ascontiguousarray(fc2_w.T)                               # [HID, C]
    dwf = dw_w.reshape(HID, 9)
    dwv = np.ascontiguousarray(dwf.reshape(MC, 128, 9))
    # order dwv taps to match TAPS ordering: column t of dwv must be TAPS[t]
    tap_idx = [(dy + 1) * 3 + (dx + 1) for (dy, dx) in TAPS]
    dwv = np.ascontiguousarray(dwv[:, :, tap_idx])
    if N_PE_TAPS:
        dgd = np.zeros((N_PE_TAPS, MC, 128, 128), np.float32)
        ii = np.arange(128)
        for t in range(N_PE_TAPS):
            dgd[t, :, ii, ii] = 0.0  # placeholder, filled below
        for t in range(N_PE_TAPS):
            for m in range(MC):
                dgd[t, m, ii, ii] = dwv[m, :, t]
    heads = np.arange(C) // CH
    seld = np.zeros((KC, 128, NH), np.float32)
    gmat = np.zeros((KC, NH, 128), np.float32)
    for c in range(KC):
        h = heads[c * 128:(c + 1) * 128]
        seld[c, np.arange(128), h] = 1.0 / CH
        gmat[c, h, np.arange(128)] = 1.0
    ins_shared = {
        "qwT": qwT, "fc1wT": fc1wT, "fc2wT": fc2wT, "dwv": dwv,
        "fc1b": np.ascontiguousarray(fc1_b.reshape(MC, 128)),
        "dwb": np.ascontiguousarray(dw_b.reshape(MC, 128)),
        "seld": seld, "gmat": gmat,
        "tempd": np.ascontiguousarray(np.repeat(temperature.reshape(NH, 1), 2, axis=1)),
        "lnwd": ln_w, "lnbd": ln_b, "fc2bd": fc2_b,
    }
    if N_PE_TAPS:
        ins_shared["dgd"] = dgd
    in_maps = [dict(ins_shared, xb=np.ascontiguousarray(x[b])) for b in range(B)]

    nc = _get_program()
    res = run_bass_kernel_spmd(nc, in_maps, list(range(B)))
    out = np.stack([res.results[b]["out"] for b in range(B)])            # [B, N, C]
    av = np.stack([res.results[b]["av"].reshape(NH, CH, N) for b in range(B)])
    return out, av
